# revision 92
# baseline (speedup 1.0000x reference)
"""GQA attention layer (B=2,S=2048,D=2048,H=16,KV=4,HD=128) on 8 trn2 cores.

Sharding: core = (b, g) for b in {0,1} (batch), g in {0..3} (kv group).
Each core computes q-heads 4g..4g+3 + kv head g for batch b, producing a
partial o-projection [S, D] (bf16); the host sums the 4 partials per batch.

Per-core kernel: transposed layout (head_dim on partitions), bf16 matmuls
with fp32 accumulation, softmax without max-subtraction (logits bounded
after RMSNorm, uniform exp bias). Key scheduling ideas:
- emission weaving: the attention pipeline (scores->exp->pv) is interleaved
  at matmul granularity with the projection stream of the next chunk and
  the o-projection of an earlier chunk, so the tensor engine never waits
  on the activation engine
- RoPE rotate-half as a DVE stream-shuffle: head_dim is host-interleaved
  so rotation pairs (m, m+64) sit on adjacent partitions (mask i^1);
  no permutation matmul on the PE
- rmsnorm restructured so the rope products (shuffle/cos/sin/add) depend
  only on the raw projection; the serial ss->ln->exp normalizer feeds a
  single final multiply -> chain latency ~2.5us, chains woven piecewise
  inside the next head's matmul loop, last head's chain deferred into the
  next emission group (no PE drain at group boundaries)
- softmax denominator accumulated in bf16 on the DVE (2x mode); a single
  ones-matmul per (chunk, head) broadcasts it across partitions
- B(3) full-tile scores run in pairs into 2-bank psum tiles (banks freed
  by retiring the A-phase pools) with ONE exp per pair, halving the ACT
  instruction count where ACT saturates
- startup: wk halves + 2-kt x pieces interleaved across the scalar+sync
  DGE rings in consumption order; PE clock-gate warmup bridges the DMA
  window; per-mc final output writes so the tail DMA overlaps
"""
import numpy as np
import ml_dtypes

B, S, DM = 2, 2048, 2048
H, KV, HD = 16, 4, 128
G = H // KV
THETA = 10000.0
EPS = 1e-6

P = 128         # partitions
CH = 512        # s-chunk (matmul N)
NCH = S // CH   # 4
KT = DM // P    # 16 contraction tiles
NST = S // P    # 16 s-tiles
EXP_BIAS = -2.0  # uniform logit shift inside exp; cancels in normalization

_CACHE = {}
# extra kwargs for run_bass_kernel_spmd (test harness sets trace/tmpdir here)
_RUN_KWARGS = {}


def _build_nc(unit_scales=True):
    from concourse import bacc, mybir, bass_isa
    import concourse.tile as tile
    from contextlib import ExitStack

    f32 = mybir.dt.float32
    bf16 = mybir.dt.bfloat16
    Act = mybir.ActivationFunctionType

    nc = bacc.Bacc()
    d_xt = nc.declare_dram_parameter("xt", [NCH, P, KT, CH], bf16, isOutput=False)
    d_wq = nc.declare_dram_parameter("wq4", [P, KT, G, HD], bf16, isOutput=False)
    d_wk = nc.declare_dram_parameter("wk1", [P, KT, HD], bf16, isOutput=False)
    d_wv = nc.declare_dram_parameter("wv1", [P, KT, HD], bf16, isOutput=False)
    d_wo = nc.declare_dram_parameter("wo4", [HD, G, DM], bf16, isOutput=False)
    d_qs = nc.declare_dram_parameter("qsc", [HD, 1], f32, isOutput=False)
    d_ks = nc.declare_dram_parameter("ksc", [HD, 1], f32, isOutput=False)
    d_cos = nc.declare_dram_parameter("cos_t", [P, S], bf16, isOutput=False)
    d_sin = nc.declare_dram_parameter("sin_t", [P, S], bf16, isOutput=False)
    d_tri = nc.declare_dram_parameter("tri", [P, P], bf16, isOutput=False)
    d_out = nc.declare_dram_parameter("o_part", [S, DM], bf16, isOutput=True)

    with tile.TileContext(nc) as tc, ExitStack() as ctx:
        const = ctx.enter_context(tc.tile_pool(name="const", bufs=1))
        xin = ctx.enter_context(tc.tile_pool(name="xin", bufs=2))
        work = ctx.enter_context(tc.tile_pool(name="work", bufs=6))
        wnorm = ctx.enter_context(tc.tile_pool(name="wnorm", bufs=4))
        defc = ctx.enter_context(tc.tile_pool(name="defc", bufs=2))
        pbp = ctx.enter_context(tc.tile_pool(name="pbp", bufs=6))
        pbp2 = ctx.enter_context(tc.tile_pool(name="pbp2", bufs=3))
        osp = ctx.enter_context(tc.tile_pool(name="osp", bufs=3))
        # PSUM, 8 banks total. Groups 1-4:
        #   p_a 2 {ps_q, ss}, p_s 3 {scores, o-proj}, p_v 1 (vps),
        #   p_cs 1 (csum broadcast), p_at 1 (attps)
        # After group 4 the A pools (p_a/p_v/p_cs, 4 banks) close and p_sp
        # (2 x 2-bank score-pair tiles) opens for B(3)'s paired exps.
        p_s = ctx.enter_context(tc.tile_pool(name="p_s", bufs=3, space="PSUM"))
        p_at = ctx.enter_context(tc.tile_pool(name="p_at", bufs=1, space="PSUM"))
        actx = ExitStack()
        p_a = actx.enter_context(tc.tile_pool(name="p_a", bufs=2, space="PSUM"))
        p_v = actx.enter_context(tc.tile_pool(name="p_v", bufs=1, space="PSUM"))
        p_cs = actx.enter_context(tc.tile_pool(name="p_cs", bufs=1, space="PSUM"))
        pools = {}

        # ---- persistent SBUF / constant loads, spread over 3 DGE rings ----
        # x chunk0 + wk + wv gate the first projection (~3MB); one ring can't
        # move that before ~19us. Interleave 2-kt x pieces across scalar+sync
        # in consumption order so the k-head can start ~11.5us and stream.
        xts = [xin.tile([P, KT, CH], bf16, tag="xt_c", name=f"xt{i}")
               for i in range(2)]
        wk_sb = const.tile([P, KT, HD], bf16, tag="wk_sb")
        wv_sb = const.tile([P, KT, HD], bf16, tag="wv_sb")
        # wk pieces ride BOTH rings (whichever ring spins up first delivers
        # early; ring startup order varies run to run). Lead with the
        # quarter-size wk[0:4] + x[kt0-1] so the first matmul starts off
        # ~384KB of arrived data instead of 768KB.
        nc.scalar.dma_start(out=wk_sb[:, 0:4], in_=d_wk[:, 0:4])
        nc.scalar.dma_start(out=xts[0][:, 0:2], in_=d_xt[0, :, 0:2])
        nc.sync.dma_start(out=xts[0][:, 2:4], in_=d_xt[0, :, 2:4])
        nc.scalar.dma_start(out=wk_sb[:, 4:8], in_=d_wk[:, 4:8])
        nc.sync.dma_start(out=wk_sb[:, 8:16], in_=d_wk[:, 8:16])
        for j, (eng, k0) in enumerate([(nc.scalar, 4), (nc.sync, 6),
                                       (nc.scalar, 8), (nc.scalar, 12),
                                       (nc.sync, 10), (nc.sync, 14)]):
            eng.dma_start(out=xts[0][:, k0:k0 + 2], in_=d_xt[0, :, k0:k0 + 2])
            if j == 1:
                nc.sync.dma_start(out=wv_sb, in_=d_wv[:])
        wq_sb = const.tile([P, KT, G, HD], bf16, tag="wq_sb")
        nc.scalar.dma_start(out=wq_sb[:, 0:4], in_=d_wq[:, 0:4])
        # cos/sin land before wq's tail: the k-head rope needs them ~17us
        cos_sb = const.tile([P, S], bf16, tag="cos_sb")
        nc.scalar.dma_start(out=cos_sb, in_=d_cos[:])
        sin_sb = const.tile([P, S], bf16, tag="sin_sb")
        nc.scalar.dma_start(out=sin_sb, in_=d_sin[:])
        for i in range(1, 4):
            nc.scalar.dma_start(out=wq_sb[:, 4 * i:4 * i + 4],
                                in_=d_wq[:, 4 * i:4 * i + 4])
        wo_sb = const.tile([P, G, DM], bf16, tag="wo_sb")
        nc.scalar.dma_start(out=wo_sb, in_=d_wo[:])
        # gpsimd ring (slow swdge): tiny tensors only
        ksc_sb = const.tile([HD, 1], f32, tag="ksc_sb")
        nc.gpsimd.dma_start(out=ksc_sb, in_=d_ks[:])
        qsc_sb = const.tile([HD, 1], f32, tag="qsc_sb")
        nc.gpsimd.dma_start(out=qsc_sb, in_=d_qs[:])
        tri_sb = const.tile([P, P], bf16, tag="tri_sb")
        nc.gpsimd.dma_start(out=tri_sb, in_=d_tri[:])

        ones_bb = const.tile([P, P], bf16, tag="ones_bb")
        nc.vector.memset(ones_bb, 1.0)
        eps_q = const.tile([P, 1], f32, tag="eps_q")
        nc.vector.memset(eps_q, float(HD * EPS))
        eps_k = const.tile([P, 1], f32, tag="eps_k")
        nc.vector.memset(eps_k, float(EPS))
        bias_e = const.tile([P, 1], f32, tag="bias_e")
        nc.vector.memset(bias_e, float(EXP_BIAS))
        # trigger the ACT table-set load now, while the PE waits on DMA
        aw = const.tile([P, 1], f32, tag="aw")
        nc.scalar.activation(out=aw, in_=eps_q, func=Act.Exp, bias=bias_e)

        # roped q heads / k / v (bf16 + fp8 copies) / normalized att
        qro = [const.tile([P, S], bf16, tag=f"qro{h}", name=f"qro{h}")
               for h in range(G)]
        kro = const.tile([P, S], bf16, tag="kro")
        v_sb = const.tile([P, NST, HD], bf16, tag="v_sb")
        att_sb = [const.tile([P, S], bf16, tag=f"att{h}", name=f"att{h}")
                  for h in range(G)]

        # warm the PE clock gate during the startup DMA wait (x chunk0 +
        # wk land ~12us in; warmup must bridge to there without overshooting)
        warm = p_s.tile([P, CH], f32, tag="sc", name="warm")
        for _ in range(42):
            nc.tensor.matmul(warm[:, :P], lhsT=ones_bb, rhs=ones_bb,
                             start=True, stop=True, skip_group_check=True)

        def prefetch_xt(c):
            for i in range(4):
                nc.sync.dma_start(out=xts[c % 2][:, 4 * i:4 * i + 4],
                                  in_=d_xt[c, :, 4 * i:4 * i + 4])

        # ---- Phase A (projections + rmsnorm + rope), one chunk ----
        # head order: k first, then the 4 q heads. The rmsnorm/rope chain of
        # head i is emitted piecewise during head i+1's matmul loop so the PE
        # never waits on the Act-engine chain. v matmuls (LDW-bound, N=128)
        # are threaded between q/k matmuls (N=512) to hide their weight loads.
        # The LAST head's chain is deferred to the next emission group
        # (gen_A_flush) so the PE never drains on the serial norm/rope chain
        # at a group boundary.
        deferred = {}

        # chain DAG (unit q/k scales; rope runs on the RAW projection, the
        # rms normalizer multiplies once at the very end, so only stage 8
        # depends on the serial ss->ln->rn path; qsq is emitted eagerly at
        # the head end as an ACT Square reading the psum directly):
        #   0 qsq=qcp^2 (DVE, legacy)  1 ss (PE)   2 ln (ACT)   3 rn (ACT)
        #   4 rsh=shuffle(qcp)  5 t1=qcp*cos  6 u=rsh*sin  7 tmp=t1+u
        #   8 dst=tmp*rn
        def emit_chain(chain, stage):
            c, h, is_q, qcp = chain["c"], chain["h"], chain["is_q"], chain["qcp"]
            cs = slice(c * CH, (c + 1) * CH)
            if stage == 0:
                qsq = wnorm.tile([P, CH], bf16, tag="qsq")
                nc.vector.tensor_mul(qsq, qcp, qcp)
                chain["qsq"] = qsq
            elif stage == 1:
                # chunk 3's flush runs after p_a closes; its ss (short-lived,
                # read only by ln) borrows the p_s ring instead
                if c == 3:
                    ss = p_s.tile([P, CH], f32, tag="sc")
                else:
                    ss = p_a.tile([P, CH], f32, tag="pa")
                nc.tensor.matmul(ss, lhsT=ones_bb, rhs=chain["qsq"],
                                 start=True, stop=True, skip_group_check=True)
                chain["ss"] = ss
            elif stage == 2:
                ln = wnorm.tile([P, CH], f32, tag="ln")
                if is_q:
                    nc.scalar.activation(out=ln, in_=chain["ss"], func=Act.Ln,
                                         scale=1.0, bias=eps_q)
                else:
                    nc.scalar.activation(out=ln, in_=chain["ss"], func=Act.Ln,
                                         scale=1.0 / HD, bias=eps_k)
                chain["ln"] = ln
            elif stage == 3:
                rn = wnorm.tile([P, CH], f32, tag="rn")
                nc.scalar.activation(out=rn, in_=chain["ln"], func=Act.Exp,
                                     scale=-0.5)
                chain["rn"] = rn
            elif stage == 4:
                # hd is host-interleaved so RoPE pairs (m, m+64) sit on
                # adjacent partitions: the rotate-half is a within-quadrant
                # DVE stream shuffle (mask i^1), not a PE matmul
                rot = work.tile([P, CH], bf16, tag="rsh")
                nc.vector.stream_shuffle(rot, qcp,
                                         mask=[i ^ 1 for i in range(32)])
                chain["rot"] = rot
            elif stage == 5:
                t1 = work.tile([P, CH], bf16, tag="t1")
                nc.vector.tensor_mul(t1, qcp, cos_sb[:, cs])
                chain["t1"] = t1
            elif stage == 6:
                u = work.tile([P, CH], bf16, tag="u")
                nc.vector.tensor_mul(u, chain["rot"], sin_sb[:, cs])
                chain["u"] = u
            elif stage == 7:
                tmp = work.tile([P, CH], bf16, tag="qs")
                nc.vector.tensor_add(tmp, chain["t1"], chain["u"])
                chain["tmp"] = tmp
            elif stage == 8:
                dst = qro[h] if is_q else kro
                nc.vector.tensor_mul(dst[:, cs], chain["tmp"], chain["rn"])
            # legacy stages (non-unit q/k scales): scale+normalize before rope
            elif stage == 14:
                qs = work.tile([P, CH], bf16, tag="qs")
                nc.vector.scalar_tensor_tensor(
                    out=qs, in0=qcp, scalar=(qsc_sb if is_q else ksc_sb),
                    in1=chain["rn"],
                    op0=mybir.AluOpType.mult, op1=mybir.AluOpType.mult)
                chain["qs"] = qs
            elif stage == 15:
                rot = work.tile([P, CH], bf16, tag="rsh")
                nc.vector.stream_shuffle(rot, chain["qs"],
                                         mask=[i ^ 1 for i in range(32)])
                chain["rot"] = rot
            elif stage == 16:
                t1 = work.tile([P, CH], bf16, tag="t1")
                nc.vector.tensor_mul(t1, chain["qs"], cos_sb[:, cs])
                u = work.tile([P, CH], bf16, tag="u")
                nc.vector.tensor_mul(u, chain["rot"], sin_sb[:, cs])
                dst = qro[h] if is_q else kro
                nc.vector.tensor_add(dst[:, cs], t1, u)

        def gen_A(c):
            xt_c = xts[c % 2]
            vps = p_v.tile([P, 4 * P], f32, tag="vps")
            vjobs = [(st, kt) for st in range(4) for kt in range(KT)]
            vi = 0
            mm = 0
            # v-jobs threaded between q/k matmuls; for chunk 0 start late so
            # the first v matmul doesn't wait on the wv DMA mid-queue
            vstart = 11 if c == 0 else 1
            fire_at = [vstart + (i * (80 - vstart)) // 64 for i in range(64)]
            chain = None  # pending norm/rope chain of the previous head

            if unit_scales:
                STAGE_AT = {1: (0,), 3: (4,), 5: (5,), 7: (6,), 9: (1,),
                            11: (2,), 13: (3, 7), 15: (8,)}
            else:
                STAGE_AT = {1: (0,), 4: (1,), 7: (2,), 9: (3,), 11: (14,),
                            13: (15,), 15: (16,)}
            for h in [G] + list(range(G)):  # k first, then q heads
                is_q = h < G
                ps_q = p_a.tile([P, CH], f32, tag="pa")
                for kt in range(KT):
                    lhs = wq_sb[:, kt, h, :] if is_q else wk_sb[:, kt, :]
                    nc.tensor.matmul(ps_q, lhsT=lhs, rhs=xt_c[:, kt],
                                     start=(kt == 0), stop=(kt == KT - 1),
                                     skip_group_check=True)
                    est = 215
                    if kt in STAGE_AT and chain is not None:
                        for stg in STAGE_AT[kt]:
                            emit_chain(chain, stg)
                        est += 150
                    if vi < 64 and fire_at[vi] <= mm:
                        st, vkt = vjobs[vi]
                        vi += 1
                        nc.tensor.matmul(vps[:, st * P:(st + 1) * P],
                                         lhsT=xt_c[:, vkt, st * P:(st + 1) * P],
                                         rhs=wv_sb[:, vkt],
                                         start=(vkt == 0), stop=(vkt == KT - 1),
                                         skip_group_check=True)
                        est += 60
                        if vkt == KT - 1:
                            stg = 4 * c + st
                            nc.vector.tensor_copy(v_sb[:, stg, :],
                                                  vps[:, st * P:(st + 1) * P])
                            est += 400
                    mm += 1
                    yield est
                # free ps_q early: snapshot the projection to SBUF (bf16).
                # The last head's snapshot lives in its own pool (defc) so
                # the deferred flush isn't WAR-serialized by the work ring.
                pool = defc if h == G - 1 else work
                qcp = pool.tile([P, CH], bf16, tag="qcp" if h == G - 1 else "qs")
                nc.vector.tensor_copy(qcp, ps_q)
                chain = {"c": c, "h": h, "is_q": is_q, "qcp": qcp}
                yield 450
            deferred[c] = chain

        def gen_A_flush(c):
            chain = deferred[c]
            if unit_scales:
                groups = ((0, 4), (1, 5), (2, 6), (3, 7), (8,))
            else:
                groups = ((0,), (1, 2), (3, 14), (15,), (16,))
            for grp in groups:
                for stg in grp:
                    emit_chain(chain, stg)
                yield 700

        # ---- Phase B (attention) for one chunk ----
        # full key tiles (t < 4c) in fp8 DoubleRow pairs; the 4 diagonal-region
        # tiles stay bf16. Consumer matmuls run one tile behind the exps.
        def gen_B(c, hs=(0, 1, 2, 3), delay=0):
            if delay:
                yield delay  # virtual delay: start late in the group
            cs = slice(c * CH, (c + 1) * CH)
            for h in hs:
                attps = p_at.tile([P, CH], f32, tag="at")
                acc = wnorm.tile([P, CH], bf16, tag="acc")
                state = {"started": False}

                def flush_one(item):
                    pb, t, off, last = item
                    nc.tensor.matmul(attps[:, off:], lhsT=v_sb[:, t, :],
                                     rhs=pb if off == 0 else pb[:, off:],
                                     start=not state["started"], stop=last,
                                     skip_group_check=True)
                    state["started"] = True

                pending = []
                npair = 2 * c if c == 3 else 0
                for jp in range(npair):
                    # full-tile PAIR: two score matmuls into the halves of a
                    # 2-bank psum tile, ONE exp over [P, 1024] (halves the
                    # ACT instruction count where ACT is the bottleneck)
                    t0 = 2 * jp
                    spair = pools["p_sp"].tile([P, 2, CH], f32, tag="sp")
                    nc.tensor.matmul(spair[:, 0, :],
                                     lhsT=kro[:, t0 * P:(t0 + 1) * P],
                                     rhs=qro[h][:, cs],
                                     start=True, stop=True,
                                     skip_group_check=True)
                    yield 900
                    nc.tensor.matmul(spair[:, 1, :],
                                     lhsT=kro[:, (t0 + 1) * P:(t0 + 2) * P],
                                     rhs=qro[h][:, cs],
                                     start=True, stop=True,
                                     skip_group_check=True)
                    pb2 = pbp2.tile([P, 2, CH], bf16, tag="pb2")
                    nc.scalar.activation(out=pb2[:, :, :], in_=spair[:, :, :],
                                         func=Act.Exp, bias=bias_e)
                    if jp == 0:
                        nc.vector.tensor_copy(acc, pb2[:, 0, :])
                    else:
                        nc.vector.tensor_add(acc, acc, pb2[:, 0, :])
                    nc.vector.tensor_add(acc, acc, pb2[:, 1, :])
                    pending.append((pb2[:, 0, :], t0, 0, False))
                    pending.append((pb2[:, 1, :], t0 + 1, 0, False))
                    while len(pending) > 2:
                        flush_one(pending.pop(0))
                    yield 900
                for t in range(2 * npair, 4 * c + 4):
                    j = t - 4 * c
                    off = P * j if j > 0 else 0
                    sc = p_s.tile([P, CH], f32, tag="sc")
                    nc.tensor.matmul(sc[:, off:], lhsT=kro[:, t * P:(t + 1) * P],
                                     rhs=qro[h][:, c * CH + off:(c + 1) * CH],
                                     start=True, stop=True, skip_group_check=True)
                    pb = pbp.tile([P, CH], bf16, tag="pb")
                    nc.scalar.activation(out=pb[:, off:], in_=sc[:, off:],
                                         func=Act.Exp, bias=bias_e)
                    if j >= 0:
                        nc.vector.tensor_mul(pb[:, off:off + P], pb[:, off:off + P],
                                             tri_sb)
                    # softmax denominator: bf16 running sum on the DVE (2x mode)
                    if t == 0:
                        nc.vector.tensor_copy(acc, pb)
                    else:
                        nc.vector.tensor_add(acc[:, off:], acc[:, off:],
                                             pb[:, off:])
                    pending.append((pb, t, off, t == 4 * c + 3))
                    if len(pending) > 2:
                        flush_one(pending.pop(0))
                    yield 900
                for it in pending:
                    flush_one(it)
                pending = []
                # normalize: one ones-matmul broadcasts the summed denominator
                if c == 3:
                    csum = p_s.tile([P, CH], f32, tag="sc", name="csum")
                else:
                    csum = p_cs.tile([P, CH], f32, tag="cs", name="csum")
                nc.tensor.matmul(csum, lhsT=ones_bb, rhs=acc,
                                 start=True, stop=True, skip_group_check=True)
                rcp = wnorm.tile([P, CH], f32, tag="rcp")
                nc.vector.reciprocal_approx_fast(out=rcp, in_=csum)
                nc.vector.tensor_mul(att_sb[h][:, cs], attps, rcp)
                yield 1500

        # ---- Phase C (output projection) for one chunk's s-tiles ----
        # half-bank psum tiles (N=256) ping-pong so the next unit's matmuls
        # overlap the previous unit's PSUM->SBUF copy
        def gen_C(c, fine_tail=False, copy_on_act=False):
            for st in range(4 * c, 4 * c + 4):
                last = fine_tail and st == 4 * c + 3
                osb = osp.tile([P, DM], bf16, tag="osb")
                for mc in range(NCH):
                    ops = p_s.tile([P, CH], f32, tag="sc")
                    for hh in range(G):
                        nc.tensor.matmul(
                            ops, lhsT=att_sb[hh][:, st * P:(st + 1) * P],
                            rhs=wo_sb[:, hh, mc * CH:(mc + 1) * CH],
                            start=(hh == 0), stop=(hh == G - 1),
                            skip_group_check=True)
                    yield 900
                    # psum->sbuf copy on ACT when its group has exp slack
                    if copy_on_act:
                        nc.scalar.copy(out=osb[:, mc * CH:(mc + 1) * CH],
                                       in_=ops)
                    else:
                        nc.vector.tensor_copy(osb[:, mc * CH:(mc + 1) * CH],
                                              ops)
                    yield 400
                    if last:
                        # final s-tile: write per-mc so the last DMA doesn't
                        # serialize a full 512KB transfer behind the last mm
                        nc.sync.dma_start(
                            out=d_out[st * P:(st + 1) * P,
                                      mc * CH:(mc + 1) * CH],
                            in_=osb[:, mc * CH:(mc + 1) * CH])
                if not last:
                    # one wide write per s-tile: 4KB DMA lines; sync engine
                    # only, so no DGE-config time lands on the Act queue
                    nc.sync.dma_start(out=d_out[st * P:(st + 1) * P, :],
                                      in_=osb)

        # ---- weaver: weighted-fair interleave of concurrent streams ----
        def run_all(*gens_weights):
            streams = [[g, float(w), 0.0] for g, w in gens_weights]
            while streams:
                s = min(streams, key=lambda x: x[2])
                try:
                    est = next(s[0])
                    s[2] += est / s[1]
                except StopIteration:
                    streams.remove(s)

        wA = 33000.0
        wC = 21000.0
        wF = 14000.0  # deferred flush: ~3500 est total -> done ~25% in

        def wB(c):
            return 4 * (2 * c * 2100 + 4 * 900 + 1300)

        prefetch_xt(1)
        run_all((gen_A(0), wA))
        prefetch_xt(2)
        run_all((gen_A_flush(0), wF), (gen_B(0), wB(0)), (gen_A(1), wA))
        prefetch_xt(3)
        run_all((gen_A_flush(1), wF), (gen_B(1), wB(1)), (gen_A(2), wA),
                (gen_C(0), wC))
        run_all((gen_A_flush(2), wF), (gen_B(2), wB(2)), (gen_A(3), wA))
        # A-phase psum pools retire; their 4 banks become B(3)'s 2-bank
        # score-pair tiles (one exp per pair of key tiles)
        actx.close()
        pools["p_sp"] = ctx.enter_context(
            tc.tile_pool(name="p_sp", bufs=2, space="PSUM"))
        run_all((gen_A_flush(3), wF), (gen_B(3), wB(3)), (gen_C(1), wC),
                (gen_C(2), wC))
        run_all((gen_C(3, fine_tail=True, copy_on_act=True), wC))

    # Pin every activation to the one table set that contains all functions
    # we use (exp/ln/square), so the ACT engine never swaps tables.
    from concourse import bacc as bacc_mod
    orig_tables = bacc_mod.get_activation_tables
    target = "natural_log_exp_and_others"

    def unified_tables(arch):
        t = orig_tables(arch)
        assert target in t
        return {k: (v if k == target else set()) for k, v in t.items()}

    bacc_mod.get_activation_tables = unified_tables
    try:
        nc.compile()
    finally:
        bacc_mod.get_activation_tables = orig_tables
    return nc


def _get_nc(unit_scales=True):
    key = ("nc", unit_scales)
    if key not in _CACHE:
        _CACHE[key] = _build_nc(unit_scales)
    return _CACHE[key]


def _rope_tables():
    inv_ts = THETA ** (-np.arange(HD // 2, dtype=np.float64) / (HD // 2))
    ang = np.arange(S, dtype=np.float64)[None, :] * inv_ts[:, None]  # [64, S]
    cos64 = np.cos(ang)
    sin64 = np.sin(ang)
    cos_t = np.concatenate([cos64, cos64], 0).astype(np.float32)
    # rotate-then-multiply signs: top rows get -sin, bottom +sin
    sin_t = np.concatenate([-sin64, sin64], 0).astype(np.float32)
    return cos_t, sin_t


def kernel(x, wq, wk, wv, wo, q_scale, k_scale):
    bf = ml_dtypes.bfloat16
    x = np.asarray(x, np.float32)
    wq = np.asarray(wq, np.float32)
    wk = np.asarray(wk, np.float32)
    wv = np.asarray(wv, np.float32)
    wo = np.asarray(wo, np.float32)
    q_scale = np.asarray(q_scale, np.float32)
    k_scale = np.asarray(k_scale, np.float32)

    from concourse.bass_utils import run_bass_kernel_spmd

    unit_scales = bool(np.allclose(q_scale, 1.0) and np.allclose(k_scale, 1.0))
    nc = _get_nc(unit_scales)
    cos_t, sin_t = _rope_tables()
    # hd interleave: RoPE pair (m, m+64) -> partitions (2m, 2m+1), so the
    # rotate-half is a within-32-quadrant shuffle on the DVE. Scores and
    # rmsnorm are invariant under this permutation of q/k head_dim.
    perm = np.empty(HD, np.int64)
    perm[0::2] = np.arange(HD // 2)
    perm[1::2] = np.arange(HD // 2) + HD // 2
    tri = (np.arange(P)[None, :] >= np.arange(P)[:, None]).astype(np.float32)

    in_maps = []
    for core in range(8):
        b, g = divmod(core, 4)
        in_maps.append({
            "xt": np.ascontiguousarray(
                x[b].T.reshape(KT, P, NCH, CH).transpose(2, 1, 0, 3)).astype(bf),
            "wq4": np.ascontiguousarray(
                wq[:, 4 * g:4 * g + 4, :][:, :, perm].reshape(
                    KT, P, G, HD).transpose(1, 0, 2, 3)).astype(bf),
            "wk1": np.ascontiguousarray(
                wk[:, g, perm].reshape(KT, P, HD).transpose(1, 0, 2)).astype(bf),
            "wv1": np.ascontiguousarray(
                wv[:, g, :].reshape(KT, P, HD).transpose(1, 0, 2)).astype(bf),
            "wo4": np.ascontiguousarray(np.transpose(wo[4 * g:4 * g + 4], (1, 0, 2))).astype(bf),
            "qsc": q_scale[perm].reshape(HD, 1),
            "ksc": k_scale[perm].reshape(HD, 1),
            "cos_t": cos_t[perm].astype(bf),
            "sin_t": sin_t[perm].astype(bf),
            "tri": tri.astype(bf),
        })

    res = run_bass_kernel_spmd(nc, in_maps, list(range(8)), **_RUN_KWARGS)
    _CACHE["last_res"] = res
    out = np.zeros((B, S, DM), np.float32)
    for core in range(8):
        out[core // 4] += np.asarray(res.results[core]["o_part"]).astype(np.float32)
    return out



# revision 93
# speedup vs baseline: 1.0060x; 1.0060x over previous
"""GQA attention layer (B=2,S=2048,D=2048,H=16,KV=4,HD=128) on 8 trn2 cores.

Sharding: core = (b, g) for b in {0,1} (batch), g in {0..3} (kv group).
Each core computes q-heads 4g..4g+3 + kv head g for batch b, producing a
partial o-projection [S, D] (bf16); the host sums the 4 partials per batch.

Per-core kernel: transposed layout (head_dim on partitions), bf16 matmuls
with fp32 accumulation, softmax without max-subtraction (logits bounded
after RMSNorm, uniform exp bias). Key scheduling ideas:
- emission weaving: the attention pipeline (scores->exp->pv) is interleaved
  at matmul granularity with the projection stream of the next chunk and
  the o-projection of an earlier chunk, so the tensor engine never waits
  on the activation engine
- RoPE rotate-half as a DVE stream-shuffle: head_dim is host-interleaved
  so rotation pairs (m, m+64) sit on adjacent partitions (mask i^1);
  no permutation matmul on the PE
- rmsnorm restructured so the rope products (shuffle/cos/sin/add) depend
  only on the raw projection; the serial ss->ln->exp normalizer feeds a
  single final multiply -> chain latency ~2.5us, chains woven piecewise
  inside the next head's matmul loop, last head's chain deferred into the
  next emission group (no PE drain at group boundaries)
- softmax denominator accumulated in bf16 on the DVE (2x mode); a single
  ones-matmul per (chunk, head) broadcasts it across partitions
- B(3) full-tile scores run in pairs into 2-bank psum tiles (banks freed
  by retiring the A-phase pools) with ONE exp per pair, halving the ACT
  instruction count where ACT saturates
- startup: wk halves + 2-kt x pieces interleaved across the scalar+sync
  DGE rings in consumption order; PE clock-gate warmup bridges the DMA
  window; per-mc final output writes so the tail DMA overlaps
"""
import numpy as np
import ml_dtypes

B, S, DM = 2, 2048, 2048
H, KV, HD = 16, 4, 128
G = H // KV
THETA = 10000.0
EPS = 1e-6

P = 128         # partitions
CH = 512        # s-chunk (matmul N)
NCH = S // CH   # 4
KT = DM // P    # 16 contraction tiles
NST = S // P    # 16 s-tiles
EXP_BIAS = -2.0  # uniform logit shift inside exp; cancels in normalization

_CACHE = {}
# extra kwargs for run_bass_kernel_spmd (test harness sets trace/tmpdir here)
_RUN_KWARGS = {}


def _build_nc(unit_scales=True):
    from concourse import bacc, mybir, bass_isa
    import concourse.tile as tile
    from contextlib import ExitStack

    f32 = mybir.dt.float32
    bf16 = mybir.dt.bfloat16
    Act = mybir.ActivationFunctionType

    nc = bacc.Bacc()
    d_xt = nc.declare_dram_parameter("xt", [NCH, P, KT, CH], bf16, isOutput=False)
    d_wq = nc.declare_dram_parameter("wq4", [P, KT, G, HD], bf16, isOutput=False)
    d_wk = nc.declare_dram_parameter("wk1", [P, KT, HD], bf16, isOutput=False)
    d_wv = nc.declare_dram_parameter("wv1", [P, KT, HD], bf16, isOutput=False)
    d_wo = nc.declare_dram_parameter("wo4", [HD, G, DM], bf16, isOutput=False)
    d_qs = nc.declare_dram_parameter("qsc", [HD, 1], f32, isOutput=False)
    d_ks = nc.declare_dram_parameter("ksc", [HD, 1], f32, isOutput=False)
    d_cos = nc.declare_dram_parameter("cos_t", [P, S], bf16, isOutput=False)
    d_sin = nc.declare_dram_parameter("sin_t", [P, S], bf16, isOutput=False)
    d_tri = nc.declare_dram_parameter("tri", [P, P], bf16, isOutput=False)
    d_out = nc.declare_dram_parameter("o_part", [S, DM], bf16, isOutput=True)

    with tile.TileContext(nc) as tc, ExitStack() as ctx:
        const = ctx.enter_context(tc.tile_pool(name="const", bufs=1))
        xin = ctx.enter_context(tc.tile_pool(name="xin", bufs=2))
        work = ctx.enter_context(tc.tile_pool(name="work", bufs=6))
        wnorm = ctx.enter_context(tc.tile_pool(name="wnorm", bufs=4))
        defc = ctx.enter_context(tc.tile_pool(name="defc", bufs=2))
        pbp = ctx.enter_context(tc.tile_pool(name="pbp", bufs=6))
        pbp2 = ctx.enter_context(tc.tile_pool(name="pbp2", bufs=3))
        osp = ctx.enter_context(tc.tile_pool(name="osp", bufs=3))
        # PSUM, 8 banks total. Groups 1-4:
        #   p_a 2 {ps_q, ss}, p_s 3 {scores, o-proj}, p_v 1 (vps),
        #   p_cs 1 (csum broadcast), p_at 1 (attps)
        # After group 4 the A pools (p_a/p_v/p_cs, 4 banks) close and p_sp
        # (2 x 2-bank score-pair tiles) opens for B(3)'s paired exps.
        p_s = ctx.enter_context(tc.tile_pool(name="p_s", bufs=3, space="PSUM"))
        p_at = ctx.enter_context(tc.tile_pool(name="p_at", bufs=1, space="PSUM"))
        actx = ExitStack()
        p_a = actx.enter_context(tc.tile_pool(name="p_a", bufs=2, space="PSUM"))
        p_v = actx.enter_context(tc.tile_pool(name="p_v", bufs=1, space="PSUM"))
        p_cs = actx.enter_context(tc.tile_pool(name="p_cs", bufs=1, space="PSUM"))
        pools = {}

        # ---- persistent SBUF / constant loads, spread over 3 DGE rings ----
        # x chunk0 + wk + wv gate the first projection (~3MB); one ring can't
        # move that before ~19us. Interleave 2-kt x pieces across scalar+sync
        # in consumption order so the k-head can start ~11.5us and stream.
        xts = [xin.tile([P, KT, CH], bf16, tag="xt_c", name=f"xt{i}")
               for i in range(2)]
        wk_sb = const.tile([P, KT, HD], bf16, tag="wk_sb")
        wv_sb = const.tile([P, KT, HD], bf16, tag="wv_sb")
        # wk pieces ride BOTH rings (whichever ring spins up first delivers
        # early; ring startup order varies run to run). Lead with the
        # quarter-size wk[0:4] + x[kt0-1] so the first matmul starts off
        # ~384KB of arrived data instead of 768KB.
        nc.scalar.dma_start(out=wk_sb[:, 0:4], in_=d_wk[:, 0:4])
        nc.scalar.dma_start(out=xts[0][:, 0:2], in_=d_xt[0, :, 0:2])
        nc.sync.dma_start(out=xts[0][:, 2:4], in_=d_xt[0, :, 2:4])
        nc.scalar.dma_start(out=wk_sb[:, 4:8], in_=d_wk[:, 4:8])
        nc.sync.dma_start(out=wk_sb[:, 8:16], in_=d_wk[:, 8:16])
        for j, (eng, k0) in enumerate([(nc.scalar, 4), (nc.sync, 6),
                                       (nc.scalar, 8), (nc.scalar, 12),
                                       (nc.sync, 10), (nc.sync, 14)]):
            eng.dma_start(out=xts[0][:, k0:k0 + 2], in_=d_xt[0, :, k0:k0 + 2])
            if j == 1:
                nc.sync.dma_start(out=wv_sb, in_=d_wv[:])
        wq_sb = const.tile([P, KT, G, HD], bf16, tag="wq_sb")
        nc.scalar.dma_start(out=wq_sb[:, 0:4], in_=d_wq[:, 0:4])
        # cos/sin land before wq's tail: the k-head rope needs them ~17us
        cos_sb = const.tile([P, S], bf16, tag="cos_sb")
        nc.scalar.dma_start(out=cos_sb, in_=d_cos[:])
        sin_sb = const.tile([P, S], bf16, tag="sin_sb")
        nc.scalar.dma_start(out=sin_sb, in_=d_sin[:])
        for i in range(1, 4):
            nc.scalar.dma_start(out=wq_sb[:, 4 * i:4 * i + 4],
                                in_=d_wq[:, 4 * i:4 * i + 4])
        wo_sb = const.tile([P, G, DM], bf16, tag="wo_sb")
        nc.scalar.dma_start(out=wo_sb, in_=d_wo[:])
        # gpsimd ring (slow swdge): tiny tensors only
        ksc_sb = const.tile([HD, 1], f32, tag="ksc_sb")
        nc.gpsimd.dma_start(out=ksc_sb, in_=d_ks[:])
        qsc_sb = const.tile([HD, 1], f32, tag="qsc_sb")
        nc.gpsimd.dma_start(out=qsc_sb, in_=d_qs[:])
        tri_sb = const.tile([P, P], bf16, tag="tri_sb")
        nc.gpsimd.dma_start(out=tri_sb, in_=d_tri[:])

        ones_bb = const.tile([P, P], bf16, tag="ones_bb")
        nc.vector.memset(ones_bb, 1.0)
        eps_q = const.tile([P, 1], f32, tag="eps_q")
        nc.vector.memset(eps_q, float(HD * EPS))
        eps_k = const.tile([P, 1], f32, tag="eps_k")
        nc.vector.memset(eps_k, float(EPS))
        bias_e = const.tile([P, 1], f32, tag="bias_e")
        nc.vector.memset(bias_e, float(EXP_BIAS))
        # trigger the ACT table-set load now, while the PE waits on DMA
        aw = const.tile([P, 1], f32, tag="aw")
        nc.scalar.activation(out=aw, in_=eps_q, func=Act.Exp, bias=bias_e)

        # roped q heads / k / v (bf16 + fp8 copies) / normalized att
        qro = [const.tile([P, S], bf16, tag=f"qro{h}", name=f"qro{h}")
               for h in range(G)]
        kro = const.tile([P, S], bf16, tag="kro")
        v_sb = const.tile([P, NST, HD], bf16, tag="v_sb")
        att_sb = [const.tile([P, S], bf16, tag=f"att{h}", name=f"att{h}")
                  for h in range(G)]

        # warm the PE clock gate during the startup DMA wait (x chunk0 +
        # wk land ~12us in; warmup must bridge to there without overshooting)
        warm = p_s.tile([P, CH], f32, tag="sc", name="warm")
        for _ in range(46):
            nc.tensor.matmul(warm[:, :P], lhsT=ones_bb, rhs=ones_bb,
                             start=True, stop=True, skip_group_check=True)

        def prefetch_xt(c):
            for i in range(4):
                nc.sync.dma_start(out=xts[c % 2][:, 4 * i:4 * i + 4],
                                  in_=d_xt[c, :, 4 * i:4 * i + 4])

        # ---- Phase A (projections + rmsnorm + rope), one chunk ----
        # head order: k first, then the 4 q heads. The rmsnorm/rope chain of
        # head i is emitted piecewise during head i+1's matmul loop so the PE
        # never waits on the Act-engine chain. v matmuls (LDW-bound, N=128)
        # are threaded between q/k matmuls (N=512) to hide their weight loads.
        # The LAST head's chain is deferred to the next emission group
        # (gen_A_flush) so the PE never drains on the serial norm/rope chain
        # at a group boundary.
        deferred = {}

        # chain DAG (unit q/k scales; rope runs on the RAW projection, the
        # rms normalizer multiplies once at the very end, so only stage 8
        # depends on the serial ss->ln->rn path; qsq is emitted eagerly at
        # the head end as an ACT Square reading the psum directly):
        #   0 qsq=qcp^2 (DVE, legacy)  1 ss (PE)   2 ln (ACT)   3 rn (ACT)
        #   4 rsh=shuffle(qcp)  5 t1=qcp*cos  6 u=rsh*sin  7 tmp=t1+u
        #   8 dst=tmp*rn
        def emit_chain(chain, stage):
            c, h, is_q, qcp = chain["c"], chain["h"], chain["is_q"], chain["qcp"]
            cs = slice(c * CH, (c + 1) * CH)
            if stage == 0:
                qsq = wnorm.tile([P, CH], bf16, tag="qsq")
                nc.vector.tensor_mul(qsq, qcp, qcp)
                chain["qsq"] = qsq
            elif stage == 1:
                # chunk 3's flush runs after p_a closes; its ss (short-lived,
                # read only by ln) borrows the p_s ring instead
                if c == 3:
                    ss = p_s.tile([P, CH], f32, tag="sc")
                else:
                    ss = p_a.tile([P, CH], f32, tag="pa")
                nc.tensor.matmul(ss, lhsT=ones_bb, rhs=chain["qsq"],
                                 start=True, stop=True, skip_group_check=True)
                chain["ss"] = ss
            elif stage == 2:
                ln = wnorm.tile([P, CH], f32, tag="ln")
                if is_q:
                    nc.scalar.activation(out=ln, in_=chain["ss"], func=Act.Ln,
                                         scale=1.0, bias=eps_q)
                else:
                    nc.scalar.activation(out=ln, in_=chain["ss"], func=Act.Ln,
                                         scale=1.0 / HD, bias=eps_k)
                chain["ln"] = ln
            elif stage == 3:
                rn = wnorm.tile([P, CH], f32, tag="rn")
                nc.scalar.activation(out=rn, in_=chain["ln"], func=Act.Exp,
                                     scale=-0.5)
                chain["rn"] = rn
            elif stage == 4:
                # hd is host-interleaved so RoPE pairs (m, m+64) sit on
                # adjacent partitions: the rotate-half is a within-quadrant
                # DVE stream shuffle (mask i^1), not a PE matmul
                rot = work.tile([P, CH], bf16, tag="rsh")
                nc.vector.stream_shuffle(rot, qcp,
                                         mask=[i ^ 1 for i in range(32)])
                chain["rot"] = rot
            elif stage == 5:
                t1 = work.tile([P, CH], bf16, tag="t1")
                nc.vector.tensor_mul(t1, qcp, cos_sb[:, cs])
                chain["t1"] = t1
            elif stage == 6:
                u = work.tile([P, CH], bf16, tag="u")
                nc.vector.tensor_mul(u, chain["rot"], sin_sb[:, cs])
                chain["u"] = u
            elif stage == 7:
                tmp = work.tile([P, CH], bf16, tag="qs")
                nc.vector.tensor_add(tmp, chain["t1"], chain["u"])
                chain["tmp"] = tmp
            elif stage == 8:
                dst = qro[h] if is_q else kro
                nc.vector.tensor_mul(dst[:, cs], chain["tmp"], chain["rn"])
            # legacy stages (non-unit q/k scales): scale+normalize before rope
            elif stage == 14:
                qs = work.tile([P, CH], bf16, tag="qs")
                nc.vector.scalar_tensor_tensor(
                    out=qs, in0=qcp, scalar=(qsc_sb if is_q else ksc_sb),
                    in1=chain["rn"],
                    op0=mybir.AluOpType.mult, op1=mybir.AluOpType.mult)
                chain["qs"] = qs
            elif stage == 15:
                rot = work.tile([P, CH], bf16, tag="rsh")
                nc.vector.stream_shuffle(rot, chain["qs"],
                                         mask=[i ^ 1 for i in range(32)])
                chain["rot"] = rot
            elif stage == 16:
                t1 = work.tile([P, CH], bf16, tag="t1")
                nc.vector.tensor_mul(t1, chain["qs"], cos_sb[:, cs])
                u = work.tile([P, CH], bf16, tag="u")
                nc.vector.tensor_mul(u, chain["rot"], sin_sb[:, cs])
                dst = qro[h] if is_q else kro
                nc.vector.tensor_add(dst[:, cs], t1, u)

        def gen_A(c):
            xt_c = xts[c % 2]
            vps = p_v.tile([P, 4 * P], f32, tag="vps")
            vjobs = [(st, kt) for st in range(4) for kt in range(KT)]
            vi = 0
            mm = 0
            # v-jobs threaded between q/k matmuls; for chunk 0 start late so
            # the first v matmul doesn't wait on the wv DMA mid-queue
            vstart = 11 if c == 0 else 1
            fire_at = [vstart + (i * (80 - vstart)) // 64 for i in range(64)]
            chain = None  # pending norm/rope chain of the previous head

            if unit_scales:
                STAGE_AT = {1: (0,), 3: (4,), 5: (5,), 7: (6,), 9: (1,),
                            11: (2,), 13: (3, 7), 15: (8,)}
            else:
                STAGE_AT = {1: (0,), 4: (1,), 7: (2,), 9: (3,), 11: (14,),
                            13: (15,), 15: (16,)}
            for h in [G] + list(range(G)):  # k first, then q heads
                is_q = h < G
                ps_q = p_a.tile([P, CH], f32, tag="pa")
                for kt in range(KT):
                    lhs = wq_sb[:, kt, h, :] if is_q else wk_sb[:, kt, :]
                    nc.tensor.matmul(ps_q, lhsT=lhs, rhs=xt_c[:, kt],
                                     start=(kt == 0), stop=(kt == KT - 1),
                                     skip_group_check=True)
                    est = 215
                    if kt in STAGE_AT and chain is not None:
                        for stg in STAGE_AT[kt]:
                            emit_chain(chain, stg)
                        est += 150
                    if vi < 64 and fire_at[vi] <= mm:
                        st, vkt = vjobs[vi]
                        vi += 1
                        nc.tensor.matmul(vps[:, st * P:(st + 1) * P],
                                         lhsT=xt_c[:, vkt, st * P:(st + 1) * P],
                                         rhs=wv_sb[:, vkt],
                                         start=(vkt == 0), stop=(vkt == KT - 1),
                                         skip_group_check=True)
                        est += 60
                        if vkt == KT - 1:
                            stg = 4 * c + st
                            nc.vector.tensor_copy(v_sb[:, stg, :],
                                                  vps[:, st * P:(st + 1) * P])
                            est += 400
                    mm += 1
                    yield est
                # free ps_q early: snapshot the projection to SBUF (bf16).
                # The last head's snapshot lives in its own pool (defc) so
                # the deferred flush isn't WAR-serialized by the work ring.
                pool = defc if h == G - 1 else work
                qcp = pool.tile([P, CH], bf16, tag="qcp" if h == G - 1 else "qs")
                nc.vector.tensor_copy(qcp, ps_q)
                chain = {"c": c, "h": h, "is_q": is_q, "qcp": qcp}
                yield 450
            deferred[c] = chain

        def gen_A_flush(c):
            chain = deferred[c]
            if unit_scales:
                groups = ((0, 4), (1, 5), (2, 6), (3, 7), (8,))
            else:
                groups = ((0,), (1, 2), (3, 14), (15,), (16,))
            for grp in groups:
                for stg in grp:
                    emit_chain(chain, stg)
                yield 700

        # ---- Phase B (attention) for one chunk ----
        # full key tiles (t < 4c) in fp8 DoubleRow pairs; the 4 diagonal-region
        # tiles stay bf16. Consumer matmuls run one tile behind the exps.
        def gen_B(c, hs=(0, 1, 2, 3), delay=0):
            if delay:
                yield delay  # virtual delay: start late in the group
            cs = slice(c * CH, (c + 1) * CH)
            for h in hs:
                attps = p_at.tile([P, CH], f32, tag="at")
                acc = wnorm.tile([P, CH], bf16, tag="acc")
                state = {"started": False}

                def flush_one(item):
                    pb, t, off, last = item
                    nc.tensor.matmul(attps[:, off:], lhsT=v_sb[:, t, :],
                                     rhs=pb if off == 0 else pb[:, off:],
                                     start=not state["started"], stop=last,
                                     skip_group_check=True)
                    state["started"] = True

                pending = []
                npair = 2 * c if c == 3 else 0
                for jp in range(npair):
                    # full-tile PAIR: two score matmuls into the halves of a
                    # 2-bank psum tile, ONE exp over [P, 1024] (halves the
                    # ACT instruction count where ACT is the bottleneck)
                    t0 = 2 * jp
                    spair = pools["p_sp"].tile([P, 2, CH], f32, tag="sp")
                    nc.tensor.matmul(spair[:, 0, :],
                                     lhsT=kro[:, t0 * P:(t0 + 1) * P],
                                     rhs=qro[h][:, cs],
                                     start=True, stop=True,
                                     skip_group_check=True)
                    yield 900
                    nc.tensor.matmul(spair[:, 1, :],
                                     lhsT=kro[:, (t0 + 1) * P:(t0 + 2) * P],
                                     rhs=qro[h][:, cs],
                                     start=True, stop=True,
                                     skip_group_check=True)
                    pb2 = pbp2.tile([P, 2, CH], bf16, tag="pb2")
                    nc.scalar.activation(out=pb2[:, :, :], in_=spair[:, :, :],
                                         func=Act.Exp, bias=bias_e)
                    if jp == 0:
                        nc.vector.tensor_copy(acc, pb2[:, 0, :])
                    else:
                        nc.vector.tensor_add(acc, acc, pb2[:, 0, :])
                    nc.vector.tensor_add(acc, acc, pb2[:, 1, :])
                    pending.append((pb2[:, 0, :], t0, 0, False))
                    pending.append((pb2[:, 1, :], t0 + 1, 0, False))
                    while len(pending) > 2:
                        flush_one(pending.pop(0))
                    yield 900
                for t in range(2 * npair, 4 * c + 4):
                    j = t - 4 * c
                    off = P * j if j > 0 else 0
                    sc = p_s.tile([P, CH], f32, tag="sc")
                    nc.tensor.matmul(sc[:, off:], lhsT=kro[:, t * P:(t + 1) * P],
                                     rhs=qro[h][:, c * CH + off:(c + 1) * CH],
                                     start=True, stop=True, skip_group_check=True)
                    pb = pbp.tile([P, CH], bf16, tag="pb")
                    nc.scalar.activation(out=pb[:, off:], in_=sc[:, off:],
                                         func=Act.Exp, bias=bias_e)
                    if j >= 0:
                        nc.vector.tensor_mul(pb[:, off:off + P], pb[:, off:off + P],
                                             tri_sb)
                    # softmax denominator: bf16 running sum on the DVE (2x mode)
                    if t == 0:
                        nc.vector.tensor_copy(acc, pb)
                    else:
                        nc.vector.tensor_add(acc[:, off:], acc[:, off:],
                                             pb[:, off:])
                    pending.append((pb, t, off, t == 4 * c + 3))
                    if len(pending) > 2:
                        flush_one(pending.pop(0))
                    yield 900
                for it in pending:
                    flush_one(it)
                pending = []
                # normalize: one ones-matmul broadcasts the summed denominator
                if c == 3:
                    csum = p_s.tile([P, CH], f32, tag="sc", name="csum")
                else:
                    csum = p_cs.tile([P, CH], f32, tag="cs", name="csum")
                nc.tensor.matmul(csum, lhsT=ones_bb, rhs=acc,
                                 start=True, stop=True, skip_group_check=True)
                rcp = wnorm.tile([P, CH], f32, tag="rcp")
                nc.vector.reciprocal_approx_fast(out=rcp, in_=csum)
                nc.vector.tensor_mul(att_sb[h][:, cs], attps, rcp)
                yield 1500

        # ---- Phase C (output projection) for one chunk's s-tiles ----
        # half-bank psum tiles (N=256) ping-pong so the next unit's matmuls
        # overlap the previous unit's PSUM->SBUF copy
        def gen_C(c, fine_tail=False, copy_on_act=False):
            for st in range(4 * c, 4 * c + 4):
                last = fine_tail and st == 4 * c + 3
                osb = osp.tile([P, DM], bf16, tag="osb")
                for mc in range(NCH):
                    ops = p_s.tile([P, CH], f32, tag="sc")
                    for hh in range(G):
                        nc.tensor.matmul(
                            ops, lhsT=att_sb[hh][:, st * P:(st + 1) * P],
                            rhs=wo_sb[:, hh, mc * CH:(mc + 1) * CH],
                            start=(hh == 0), stop=(hh == G - 1),
                            skip_group_check=True)
                    yield 900
                    # psum->sbuf copy on ACT when its group has exp slack
                    if copy_on_act:
                        nc.scalar.copy(out=osb[:, mc * CH:(mc + 1) * CH],
                                       in_=ops)
                    else:
                        nc.vector.tensor_copy(osb[:, mc * CH:(mc + 1) * CH],
                                              ops)
                    yield 400
                    if last:
                        # final s-tile: write per-mc so the last DMA doesn't
                        # serialize a full 512KB transfer behind the last mm
                        nc.sync.dma_start(
                            out=d_out[st * P:(st + 1) * P,
                                      mc * CH:(mc + 1) * CH],
                            in_=osb[:, mc * CH:(mc + 1) * CH])
                if not last:
                    # one wide write per s-tile: 4KB DMA lines; sync engine
                    # only, so no DGE-config time lands on the Act queue
                    nc.sync.dma_start(out=d_out[st * P:(st + 1) * P, :],
                                      in_=osb)

        # ---- weaver: weighted-fair interleave of concurrent streams ----
        def run_all(*gens_weights):
            streams = [[g, float(w), 0.0] for g, w in gens_weights]
            while streams:
                s = min(streams, key=lambda x: x[2])
                try:
                    est = next(s[0])
                    s[2] += est / s[1]
                except StopIteration:
                    streams.remove(s)

        wA = 33000.0
        wC = 21000.0
        wF = 14000.0  # deferred flush: ~3500 est total -> done ~25% in

        def wB(c):
            return 4 * (2 * c * 2100 + 4 * 900 + 1300)

        prefetch_xt(1)
        run_all((gen_A(0), wA))
        prefetch_xt(2)
        run_all((gen_A_flush(0), wF), (gen_B(0), wB(0)), (gen_A(1), wA))
        prefetch_xt(3)
        run_all((gen_A_flush(1), wF), (gen_B(1), wB(1)), (gen_A(2), wA),
                (gen_C(0), wC))
        run_all((gen_A_flush(2), wF), (gen_B(2), wB(2)), (gen_A(3), wA))
        # A-phase psum pools retire; their 4 banks become B(3)'s 2-bank
        # score-pair tiles (one exp per pair of key tiles)
        actx.close()
        pools["p_sp"] = ctx.enter_context(
            tc.tile_pool(name="p_sp", bufs=2, space="PSUM"))
        run_all((gen_A_flush(3), wF), (gen_B(3), wB(3)), (gen_C(1), wC),
                (gen_C(2), wC))
        run_all((gen_C(3, fine_tail=True, copy_on_act=True), wC))

    # Pin every activation to the one table set that contains all functions
    # we use (exp/ln/square), so the ACT engine never swaps tables.
    from concourse import bacc as bacc_mod
    orig_tables = bacc_mod.get_activation_tables
    target = "natural_log_exp_and_others"

    def unified_tables(arch):
        t = orig_tables(arch)
        assert target in t
        return {k: (v if k == target else set()) for k, v in t.items()}

    bacc_mod.get_activation_tables = unified_tables
    try:
        nc.compile()
    finally:
        bacc_mod.get_activation_tables = orig_tables
    return nc


def _get_nc(unit_scales=True):
    key = ("nc", unit_scales)
    if key not in _CACHE:
        _CACHE[key] = _build_nc(unit_scales)
    return _CACHE[key]


def _rope_tables():
    inv_ts = THETA ** (-np.arange(HD // 2, dtype=np.float64) / (HD // 2))
    ang = np.arange(S, dtype=np.float64)[None, :] * inv_ts[:, None]  # [64, S]
    cos64 = np.cos(ang)
    sin64 = np.sin(ang)
    cos_t = np.concatenate([cos64, cos64], 0).astype(np.float32)
    # rotate-then-multiply signs: top rows get -sin, bottom +sin
    sin_t = np.concatenate([-sin64, sin64], 0).astype(np.float32)
    return cos_t, sin_t


def kernel(x, wq, wk, wv, wo, q_scale, k_scale):
    bf = ml_dtypes.bfloat16
    x = np.asarray(x, np.float32)
    wq = np.asarray(wq, np.float32)
    wk = np.asarray(wk, np.float32)
    wv = np.asarray(wv, np.float32)
    wo = np.asarray(wo, np.float32)
    q_scale = np.asarray(q_scale, np.float32)
    k_scale = np.asarray(k_scale, np.float32)

    from concourse.bass_utils import run_bass_kernel_spmd

    unit_scales = bool(np.allclose(q_scale, 1.0) and np.allclose(k_scale, 1.0))
    nc = _get_nc(unit_scales)
    cos_t, sin_t = _rope_tables()
    # hd interleave: RoPE pair (m, m+64) -> partitions (2m, 2m+1), so the
    # rotate-half is a within-32-quadrant shuffle on the DVE. Scores and
    # rmsnorm are invariant under this permutation of q/k head_dim.
    perm = np.empty(HD, np.int64)
    perm[0::2] = np.arange(HD // 2)
    perm[1::2] = np.arange(HD // 2) + HD // 2
    tri = (np.arange(P)[None, :] >= np.arange(P)[:, None]).astype(np.float32)

    in_maps = []
    for core in range(8):
        b, g = divmod(core, 4)
        in_maps.append({
            "xt": np.ascontiguousarray(
                x[b].T.reshape(KT, P, NCH, CH).transpose(2, 1, 0, 3)).astype(bf),
            "wq4": np.ascontiguousarray(
                wq[:, 4 * g:4 * g + 4, :][:, :, perm].reshape(
                    KT, P, G, HD).transpose(1, 0, 2, 3)).astype(bf),
            "wk1": np.ascontiguousarray(
                wk[:, g, perm].reshape(KT, P, HD).transpose(1, 0, 2)).astype(bf),
            "wv1": np.ascontiguousarray(
                wv[:, g, :].reshape(KT, P, HD).transpose(1, 0, 2)).astype(bf),
            "wo4": np.ascontiguousarray(np.transpose(wo[4 * g:4 * g + 4], (1, 0, 2))).astype(bf),
            "qsc": q_scale[perm].reshape(HD, 1),
            "ksc": k_scale[perm].reshape(HD, 1),
            "cos_t": cos_t[perm].astype(bf),
            "sin_t": sin_t[perm].astype(bf),
            "tri": tri.astype(bf),
        })

    res = run_bass_kernel_spmd(nc, in_maps, list(range(8)), **_RUN_KWARGS)
    _CACHE["last_res"] = res
    out = np.zeros((B, S, DM), np.float32)
    for core in range(8):
        out[core // 4] += np.asarray(res.results[core]["o_part"]).astype(np.float32)
    return out

